# revision 24
# baseline (speedup 1.0000x reference)
"""Trainium2 Bass kernel for nn_CrossModalAttention (sparse per-channel 3x3
token-window attention).

Contract: kernel(**inputs) takes the FULL fp32 inputs (B=8,C=256,H=W=64) and
returns the FULL fp32 output.  Internally: data-parallel over batch across the
8 NeuronCores (1 batch element per core), params replicated.

v3 design (calibrated against the fake-NRT cost model):
  - K projection runs as fp8e4 DoubleRow matmuls: one 256-deep contraction
    pass per 512-pixel chunk (half the PE cycles of bf16).  Weights are
    pre-scaled by 16 on the host to dodge fp8 subnormals; the PSUM evacuation
    un-scales (ACT activation scale=1/16) and fuses the bias + bf16 cast.
    Q and V stay bf16 (V errors hit the output linearly; Q feeds the
    residual-critical logits).
  - QK/AV elementwise products are bf16 TensorTensor ops (the only DVE op
    with a 2x mode here), split between DVE and GpSimd(Pool): Pool takes the
    late-consumed neighbors so it never sits on the softmax critical path.
  - PE reduces products with identity-matmul accumulation chains; the 9-way
    softmax denominator is also a PE identity chain into PSUM.
  - reciprocal_approx_fast on DVE; one fat 2x e*(1/s) multiply per group.
  - Residual add + store in bf16 (host casts back to fp32).

Layout: activations are d-major [c, d, t]; d = pixel-within-token (16),
t = (I,J) token index (256).  K/V live in zero-padded 18x18 token grids so
all 9 neighbor views are plain strided APs (this simulator's DVE 2x mode has
no offset-parity constraint).
"""

import os
import sys
from contextlib import ExitStack

import numpy as np

for _p in ("/opt/trn_rl_repo",):
    if _p not in sys.path and os.path.isdir(_p):
        sys.path.insert(0, _p)

import ml_dtypes  # noqa: E402

import concourse.bacc as bacc  # noqa: E402
import concourse.bass as bass  # noqa: E402
import concourse.tile as tile  # noqa: E402
from concourse import mybir  # noqa: E402
from concourse.bass_utils import run_bass_kernel_spmd  # noqa: E402

BF16 = mybir.dt.bfloat16
F32 = mybir.dt.float32
FP8 = mybir.dt.float8e4
ALU = mybir.AluOpType
ACTF = mybir.ActivationFunctionType
DR = mybir.MatmulPerfMode.DoubleRow

B, C, H, W = 8, 256, 64, 64
TS = 4                      # token size
NH = H // TS                # 16 token rows
NW = W // TS                # 16 token cols
T = NH * NW                 # 256 tokens
D = TS * TS                 # 16 pixels per token
G = 2                       # channel groups of 128
P = 128
PIX = H * W                 # 4096
GRID = NH + 2               # 18 (zero-padded token grid)
SCALE = float(D) ** -0.5    # 0.25
N_CORES = 8
WSCALE = 16.0               # fp8 weight pre-scale (undone at evacuation)

# smalls layout (bf16): ident | Wq lhsT | Wk lhsT | Wv lhsT | biases
SM_IDENT = 0
SM_WQ = P                       # (g,h) blocks of 128 -> 512
SM_WK = SM_WQ + 4 * P
SM_WV = SM_WK + 4 * P
SM_BALL = SM_WV + 4 * P         # 6 cols (proj*2+g, proj order Q,K,V)
SMW = SM_BALL + 6

PAIRS = ((8,), (0, 1), (2, 3), (4, 5), (6, 7))
POOL_QK = {7, 8}      # QK neighbor products emitted on Pool (both groups)
POOL_AV = {0: {7, 8}, 1: {8}}   # per-half AV pool assignment

_BUILT = None


def _emit(ctx: ExitStack, tc: "tile.TileContext"):
    nc = tc.nc

    xb_d = nc.dram_tensor("xb", [P, G, PIX], BF16, kind="ExternalInput").ap()
    xw_d = nc.dram_tensor("xw", [P, G, PIX], BF16, kind="ExternalInput").ap()
    sm_d = nc.dram_tensor("sm", [P, SMW], BF16, kind="ExternalInput").ap()
    out_d = nc.dram_tensor("out", [P, G, PIX], BF16, kind="ExternalOutput").ap()

    consts = ctx.enter_context(tc.tile_pool(name="consts", bufs=1))
    php = ctx.enter_context(tc.tile_pool(name="php", bufs=4))
    avp = ctx.enter_context(tc.tile_pool(name="avp", bufs=11))
    enhp = ctx.enter_context(tc.tile_pool(name="enhp", bufs=2))
    outp = ctx.enter_context(tc.tile_pool(name="outp", bufs=2))
    psP = ctx.enter_context(tc.tile_pool(name="psP", bufs=2, space="PSUM"))
    psL = ctx.enter_context(tc.tile_pool(name="psL", bufs=2, space="PSUM"))
    psA = ctx.enter_context(tc.tile_pool(name="psA", bufs=1, space="PSUM"))

    xb = consts.tile([P, G, PIX], BF16)
    xw = consts.tile([P, G, PIX], BF16)
    sm = consts.tile([P, SMW], BF16)
    qf = consts.tile([P, G, D, T], BF16)
    kvp = consts.tile([P, 2, G, D, GRID, GRID], BF16)  # [kv, g, d, I, J]
    eslab = consts.tile([P, G, 9, T], BF16)
    rr = consts.tile([P, G, T], BF16)
    rrf = consts.tile([P, G, T], F32)
    ssb = consts.tile([P, G, T], F32)

    nc.sync.dma_start(sm[:], sm_d[:])
    nc.sync.dma_start(xb[:], xb_d[:])
    nc.scalar.dma_start(xw[:], xw_d[:])

    ident = sm[:, SM_IDENT:SM_IDENT + P]
    wq = sm[:, SM_WQ:SM_WQ + 4 * P].rearrange("p (g h c) -> p g h c", g=G, h=2)
    wk = sm[:, SM_WK:SM_WK + 4 * P].rearrange("p (g h c) -> p g h c", g=G, h=2)
    wv = sm[:, SM_WV:SM_WV + 4 * P].rearrange("p (g h c) -> p g h c", g=G, h=2)
    ball = sm[:, SM_BALL:SM_BALL + 6]

    # zero the padding ring of the K/V token grids (rows and cols 0,17).
    # Pool is idle at program start; these have no dependencies.
    for kv in range(2):
        for g in range(G):
            nc.gpsimd.memset(kvp[:, kv, g, :, 0:GRID:GRID - 1, :], 0.0)
            nc.gpsimd.memset(kvp[:, kv, g, :, :, 0:GRID:GRID - 1], 0.0)

    # ---------------- projection emitters ----------------
    def proj_unit(proj, g, u):
        """one 1024-pixel chunk of a bf16 projection + evacuation.
        proj: 0=Q (from xb -> qf), 1=K (xw -> grid 0), 2=V (xw -> grid 1)."""
        w = (wq, wk, wv)[proj]
        src = xb if proj == 0 else xw
        bias = ball[:, 2 * proj + g:2 * proj + g + 1]
        pt = psP.tile([P, 1024], F32, tag="psP")
        for j in range(2):
            cols = slice(u * 1024 + j * 512, u * 1024 + (j + 1) * 512)
            nc.tensor.matmul(pt[:, j * 512:(j + 1) * 512], w[:, g, 0],
                             src[:, 0, cols], start=True, stop=False)
            nc.tensor.matmul(pt[:, j * 512:(j + 1) * 512], w[:, g, 1],
                             src[:, 1, cols], start=False, stop=True)
        if proj == 0:
            nc.scalar.activation(qf[:, g, 4 * u:4 * u + 4, :], pt[:],
                                 ACTF.Identity, bias=bias)
        else:
            pv = pt[:].rearrange("p (d i j) -> p d i j", d=4, i=NH)
            nc.scalar.activation(
                kvp[:, proj - 1, g, 4 * u:4 * u + 4, 1:1 + NH, 1:1 + NW], pv,
                ACTF.Identity, bias=bias)

    # ---------------- attention emitters ----------------
    def qk_pair(g, pair):
        """products (per-neighbor tiles) + PE d-reduction chain + exp"""
        tiles = []
        for n in pair:
            di, dj = n // 3, n % 3
            ph = php.tile([P, D, T], BF16, tag="php")
            eng = nc.gpsimd if n in POOL_QK else nc.vector
            eng.tensor_tensor(ph[:], qf[:, g],
                              kvp[:, 0, g, :, di:di + NH, dj:dj + NW],
                              op=ALU.mult)
            tiles.append(ph)
        lp = psL.tile([P, 512], F32, tag="psL")
        for w, ph in enumerate(tiles):
            for d in range(D):
                nc.tensor.matmul(lp[:, w * T:(w + 1) * T], ident,
                                 ph[:, d, :], start=(d == 0),
                                 stop=(d == D - 1))
        nc.scalar.activation(eslab[:, g, pair[0]:pair[0] + len(pair), :],
                             lp[:, :len(pair) * T], ACTF.Exp, scale=SCALE)

    def softmax_sums(g):
        """9-way sum on PE into PSUM, evacuated to SBUF by ACT"""
        sp = psL.tile([P, 512], F32, tag="psL")
        for n in range(9):
            nc.tensor.matmul(sp[:, :T], ident, eslab[:, g, n, :],
                             start=(n == 0), stop=(n == 8))
        nc.scalar.copy(ssb[:, g, :], sp[:, :T])

    def softmax_dve(g):
        """approx reciprocal, e <- e * (1/s)"""
        nc.vector.reciprocal_approx_fast(rrf[:, g, :], ssb[:, g, :])
        nc.scalar.copy(rr[:, g, :], rrf[:, g, :])
        ev = eslab[:, g]
        nc.vector.tensor_tensor(
            ev, ev, rr[:, g, :].unsqueeze(1).broadcast_to([P, 9, T]),
            op=ALU.mult)

    def av_products(g, hf):
        """p*V products for one d-half, per-neighbor tiles"""
        ds = slice(8 * hf, 8 * hf + 8)
        tiles = {}
        for n in list(range(9)):
            di, dj = n // 3, n % 3
            av = avp.tile([P, D // 2, T], BF16, tag="avs")
            pe = eslab[:, g, n, :].unsqueeze(1).broadcast_to([P, 8, T])
            eng = nc.gpsimd if n in POOL_AV[hf] else nc.vector
            eng.tensor_tensor(av[:], kvp[:, 1, g, ds, di:di + NH, dj:dj + NW],
                              pe, op=ALU.mult)
            tiles[n] = av
        return tiles

    def av_reduce(g, hf, tiles, enh):
        """accumulate over neighbors on PE in 4-d-plane quarters"""
        for q in range(2):
            acc = psA.tile([P, 1024], F32, tag="psA")
            for it, (n, av) in enumerate(sorted(tiles.items())):
                fl = av[:].rearrange("p d t -> p (d t)")
                for j in range(2):
                    cs = slice(q * 1024 + j * 512, q * 1024 + (j + 1) * 512)
                    nc.tensor.matmul(acc[:, j * 512:(j + 1) * 512], ident,
                                     fl[:, cs], start=(it == 0),
                                     stop=(it == 8))
            nc.scalar.activation(enh[:, 8 * hf + 4 * q:8 * hf + 4 * q + 4, :],
                                 acc[:], ACTF.Identity)

    def tail(g, enh, hf):
        cs = slice(hf * 2048, (hf + 1) * 2048)
        ot = outp.tile([P, PIX // 2], BF16, tag="outf")
        nc.vector.tensor_tensor(
            ot[:], enh[:].rearrange("p d t -> p (d t)")[:, cs],
            xb[:, g, cs], op=ALU.add)
        nc.sync.dma_start(out_d[:, g, cs], ot[:])

    # ---------------- schedule ----------------
    # group 0 Q and K projections first (QK products gate on them),
    # interleaved by chunk so both finish as early as the DMA allows
    for u in range(4):
        proj_unit(0, 0, u)
        proj_unit(1, 0, u)

    # remaining projections interleaved into the QK g0 PE stream:
    # Q1/K1 early (QK g1 products follow right after QK g0 on DVE), then V0, V1
    rest = ([(0, 1, u) for u in range(4)] + [(1, 1, u) for u in range(4)]
            + [(2, 0, u) for u in range(4)] + [(2, 1, u) for u in range(4)])

    splits = (0, 6, 12, 16, 16, 16)
    for i, pair in enumerate(PAIRS):
        qk_pair(0, pair)
        for pr, g, u in rest[splits[i]:splits[i + 1]]:
            proj_unit(pr, g, u)
    softmax_sums(0)
    for pair in PAIRS:
        qk_pair(1, pair)
    softmax_dve(0)
    softmax_sums(1)

    enh0 = enhp.tile([P, D, T], BF16, tag="enh")
    for hf in range(2):
        t0 = av_products(0, hf)
        av_reduce(0, hf, t0, enh0)
        tail(0, enh0, hf)
        if hf == 0:
            softmax_dve(1)
    enh1 = enhp.tile([P, D, T], BF16, tag="enh")
    for hf in range(2):
        t1 = av_products(1, hf)
        av_reduce(1, hf, t1, enh1)
        tail(1, enh1, hf)


def _build():
    global _BUILT
    if _BUILT is None:
        nc = bacc.Bacc(
            "TRN2", target_bir_lowering=False, debug=False, num_devices=N_CORES
        )
        with tile.TileContext(nc) as tc:
            with ExitStack() as ctx:
                _emit(ctx, tc)
        nc.compile()
        _BUILT = nc
    return _BUILT


def _tokenize(x: np.ndarray) -> np.ndarray:
    """[C,H,W] -> [C, D*T] d-major token order: index = (u,v,I,J)."""
    c = x.shape[0]
    return (
        x.reshape(c, NH, TS, NW, TS).transpose(0, 2, 4, 1, 3).reshape(c, PIX)
    )


def _untokenize(y: np.ndarray) -> np.ndarray:
    """[C, D*T] d-major token order -> [C, H, W]."""
    c = y.shape[0]
    return (
        y.reshape(c, TS, TS, NH, NW).transpose(0, 3, 1, 4, 2).reshape(c, H, W)
    )


def _part_fold(x: np.ndarray) -> np.ndarray:
    """[C, F] -> [P, C//P, F] partition-major fold."""
    return np.ascontiguousarray(
        x.reshape(C // P, P, -1).transpose(1, 0, 2)
    )


def _lhsT_blocks(wmat, dst, base, scale=1.0):
    """pack W[c_out, a_in] into per-(g,h) lhsT blocks: lhsT[p, c]=W[gP+c, hP+p]"""
    wm = np.asarray(wmat, np.float32) * scale
    for g in range(G):
        for h in range(2):
            blk = wm[g * P:(g + 1) * P, h * P:(h + 1) * P].T
            o = base + (g * 2 + h) * P
            dst[:, o:o + P] = blk


def _prep_maps(blue_feat, white_feat, Wq, bq, Wk, bk, Wv, bv):
    bf16 = ml_dtypes.bfloat16
    fp8 = mybir.dt.np(FP8)

    sm = np.zeros((P, SMW), np.float32)
    sm[:, SM_IDENT:SM_IDENT + P] = np.eye(P, dtype=np.float32)
    _lhsT_blocks(Wq, sm, SM_WQ)
    _lhsT_blocks(Wk, sm, SM_WK)
    _lhsT_blocks(Wv, sm, SM_WV)
    for g in range(G):
        sm[:, SM_BALL + g] = np.asarray(bq, np.float32)[g * P:(g + 1) * P]
        sm[:, SM_BALL + 2 + g] = np.asarray(bk, np.float32)[g * P:(g + 1) * P]
        sm[:, SM_BALL + 4 + g] = np.asarray(bv, np.float32)[g * P:(g + 1) * P]
    sm = sm.astype(bf16)

    maps = []
    for b in range(B):
        xbm = _part_fold(_tokenize(np.asarray(blue_feat[b], np.float32)))
        xwm = _part_fold(_tokenize(np.asarray(white_feat[b], np.float32)))
        maps.append({
            "xb": xbm.astype(bf16),
            "xw": xwm.astype(bf16),
            "sm": sm,
        })
    return maps


def _gather(results) -> np.ndarray:
    out = np.empty((B, C, H, W), np.float32)
    for b in range(B):
        y = results[b]["out"]  # [P, G, PIX] bf16
        y = np.asarray(y, np.float32).transpose(1, 0, 2).reshape(C, PIX)
        out[b] = _untokenize(y)
    return out


def _install_ntff_hook():
    """The agent image's antenv lacks axon_hooks; synthesize it so
    run_bass_kernel_spmd(trace=True) can drive NTFF profiling via the
    injected libaxon_pjrt.so C ABI (mirrors trn_agent_boot.trn_boot)."""
    import contextlib
    import ctypes
    import types

    if "antenv.axon_hooks" in sys.modules:
        return
    so_path = "/opt/axon/libaxon_pjrt.so"
    lib = ctypes.CDLL(so_path)
    if not hasattr(lib, "axon_start_nrt_profile"):
        return
    lib.axon_start_nrt_profile.argtypes = [
        ctypes.POINTER(ctypes.c_int64),
        ctypes.c_size_t,
    ]
    lib.axon_start_nrt_profile.restype = ctypes.c_int64
    lib.axon_stop_nrt_profile.argtypes = [ctypes.c_char_p]
    lib.axon_stop_nrt_profile.restype = ctypes.c_int64

    @contextlib.contextmanager
    def _hook(output_dir, device_ids):
        import jax

        jax.devices()
        if device_ids:
            ids = (ctypes.c_int64 * len(device_ids))(*device_ids)
            rc = lib.axon_start_nrt_profile(ids, len(device_ids))
        else:
            rc = lib.axon_start_nrt_profile(None, 0)
        if rc != 0:
            raise RuntimeError(f"axon_start_nrt_profile rc={rc}")
        try:
            yield
        finally:
            n = lib.axon_stop_nrt_profile(str(output_dir).encode())
            print(f"ntff profile: {n} file(s) written to {output_dir}")

    mod = types.ModuleType("antenv.axon_hooks")
    mod.get_axon_ntff_profile_hook = lambda: _hook  # type: ignore[attr-defined]
    mod.set_axon_ntff_profile_hook = lambda h: None  # type: ignore[attr-defined]
    sys.modules["antenv.axon_hooks"] = mod


def run(trace=False, **inputs):
    nc = _build()
    maps = _prep_maps(**inputs)
    if trace:
        _install_ntff_hook()
    res = run_bass_kernel_spmd(nc, maps, list(range(N_CORES)), trace=trace)
    return _gather(res.results), res


def kernel(**inputs) -> np.ndarray:
    out, _ = run(trace=False, **inputs)
    return out


# revision 25
# speedup vs baseline: 1.0292x; 1.0292x over previous
"""Trainium2 Bass kernel for nn_CrossModalAttention (sparse per-channel 3x3
token-window attention).

Contract: kernel(**inputs) takes the FULL fp32 inputs (B=8,C=256,H=W=64) and
returns the FULL fp32 output.  Internally: data-parallel over batch across the
8 NeuronCores (1 batch element per core), params replicated.

v3 design (calibrated against the fake-NRT cost model):
  - K projection runs as fp8e4 DoubleRow matmuls: one 256-deep contraction
    pass per 512-pixel chunk (half the PE cycles of bf16).  Weights are
    pre-scaled by 16 on the host to dodge fp8 subnormals; the PSUM evacuation
    un-scales (ACT activation scale=1/16) and fuses the bias + bf16 cast.
    Q and V stay bf16 (V errors hit the output linearly; Q feeds the
    residual-critical logits).
  - QK/AV elementwise products are bf16 TensorTensor ops (the only DVE op
    with a 2x mode here), split between DVE and GpSimd(Pool): Pool takes the
    late-consumed neighbors so it never sits on the softmax critical path.
  - PE reduces products with identity-matmul accumulation chains; the 9-way
    softmax denominator is also a PE identity chain into PSUM.
  - reciprocal_approx_fast on DVE; one fat 2x e*(1/s) multiply per group.
  - Residual add + store in bf16 (host casts back to fp32).

Layout: activations are d-major [c, d, t]; d = pixel-within-token (16),
t = (I,J) token index (256).  K/V live in zero-padded 18x18 token grids so
all 9 neighbor views are plain strided APs (this simulator's DVE 2x mode has
no offset-parity constraint).
"""

import os
import sys
from contextlib import ExitStack

import numpy as np

for _p in ("/opt/trn_rl_repo",):
    if _p not in sys.path and os.path.isdir(_p):
        sys.path.insert(0, _p)

import ml_dtypes  # noqa: E402

import concourse.bacc as bacc  # noqa: E402
import concourse.bass as bass  # noqa: E402
import concourse.tile as tile  # noqa: E402
from concourse import mybir  # noqa: E402
from concourse.bass_utils import run_bass_kernel_spmd  # noqa: E402

BF16 = mybir.dt.bfloat16
F32 = mybir.dt.float32
FP8 = mybir.dt.float8e4
ALU = mybir.AluOpType
ACTF = mybir.ActivationFunctionType
DR = mybir.MatmulPerfMode.DoubleRow

B, C, H, W = 8, 256, 64, 64
TS = 4                      # token size
NH = H // TS                # 16 token rows
NW = W // TS                # 16 token cols
T = NH * NW                 # 256 tokens
D = TS * TS                 # 16 pixels per token
G = 2                       # channel groups of 128
P = 128
PIX = H * W                 # 4096
GRID = NH + 2               # 18 (zero-padded token grid)
SCALE = float(D) ** -0.5    # 0.25
N_CORES = 8
WSCALE = 16.0               # fp8 weight pre-scale (undone at evacuation)

# smalls layout (bf16): ident | Wq lhsT | Wk lhsT | Wv lhsT | biases
SM_IDENT = 0
SM_WQ = P                       # (g,h) blocks of 128 -> 512
SM_WK = SM_WQ + 4 * P
SM_WV = SM_WK + 4 * P
SM_BALL = SM_WV + 4 * P         # 6 cols (proj*2+g, proj order Q,K,V)
SMW = SM_BALL + 6

PAIRS = ((8,), (0, 1), (2, 3), (4, 5), (6, 7))
POOL_QK = {8}         # QK neighbor products emitted on Pool (both groups)
POOL_AV = {0: {7, 8}, 1: {8}}   # per-half AV pool assignment

_BUILT = None


def _emit(ctx: ExitStack, tc: "tile.TileContext"):
    nc = tc.nc

    xb_d = nc.dram_tensor("xb", [P, G, PIX], BF16, kind="ExternalInput").ap()
    xw_d = nc.dram_tensor("xw", [P, G, PIX], BF16, kind="ExternalInput").ap()
    sm_d = nc.dram_tensor("sm", [P, SMW], BF16, kind="ExternalInput").ap()
    out_d = nc.dram_tensor("out", [P, G, PIX], BF16, kind="ExternalOutput").ap()

    consts = ctx.enter_context(tc.tile_pool(name="consts", bufs=1))
    php = ctx.enter_context(tc.tile_pool(name="php", bufs=4))
    avp = ctx.enter_context(tc.tile_pool(name="avp", bufs=11))
    enhp = ctx.enter_context(tc.tile_pool(name="enhp", bufs=2))
    outp = ctx.enter_context(tc.tile_pool(name="outp", bufs=2))
    psP = ctx.enter_context(tc.tile_pool(name="psP", bufs=2, space="PSUM"))
    psL = ctx.enter_context(tc.tile_pool(name="psL", bufs=2, space="PSUM"))
    psA = ctx.enter_context(tc.tile_pool(name="psA", bufs=1, space="PSUM"))

    xb = consts.tile([P, G, PIX], BF16)
    xw = consts.tile([P, G, PIX], BF16)
    sm = consts.tile([P, SMW], BF16)
    qf = consts.tile([P, G, D, T], BF16)
    kvp = consts.tile([P, 2, G, D, GRID, GRID], BF16)  # [kv, g, d, I, J]
    eslab = consts.tile([P, G, 9, T], BF16)
    rr = consts.tile([P, G, T], BF16)
    rrf = consts.tile([P, G, T], F32)
    ssb = consts.tile([P, G, T], F32)

    nc.sync.dma_start(sm[:], sm_d[:])
    nc.sync.dma_start(xb[:], xb_d[:])
    nc.scalar.dma_start(xw[:], xw_d[:])

    ident = sm[:, SM_IDENT:SM_IDENT + P]
    wq = sm[:, SM_WQ:SM_WQ + 4 * P].rearrange("p (g h c) -> p g h c", g=G, h=2)
    wk = sm[:, SM_WK:SM_WK + 4 * P].rearrange("p (g h c) -> p g h c", g=G, h=2)
    wv = sm[:, SM_WV:SM_WV + 4 * P].rearrange("p (g h c) -> p g h c", g=G, h=2)
    ball = sm[:, SM_BALL:SM_BALL + 6]

    # zero the padding ring of the K/V token grids (rows and cols 0,17).
    for kv in range(2):
        for g in range(G):
            nc.vector.memset(kvp[:, kv, g, :, 0:GRID:GRID - 1, :], 0.0)
            nc.vector.memset(kvp[:, kv, g, :, :, 0:GRID:GRID - 1], 0.0)

    # ---------------- projection emitters ----------------
    def proj_unit(proj, g, u):
        """one 1024-pixel chunk of a bf16 projection + evacuation.
        proj: 0=Q (from xb -> qf), 1=K (xw -> grid 0), 2=V (xw -> grid 1)."""
        w = (wq, wk, wv)[proj]
        src = xb if proj == 0 else xw
        bias = ball[:, 2 * proj + g:2 * proj + g + 1]
        pt = psP.tile([P, 1024], F32, tag="psP")
        for j in range(2):
            cols = slice(u * 1024 + j * 512, u * 1024 + (j + 1) * 512)
            nc.tensor.matmul(pt[:, j * 512:(j + 1) * 512], w[:, g, 0],
                             src[:, 0, cols], start=True, stop=False)
            nc.tensor.matmul(pt[:, j * 512:(j + 1) * 512], w[:, g, 1],
                             src[:, 1, cols], start=False, stop=True)
        if proj == 0:
            nc.scalar.activation(qf[:, g, 4 * u:4 * u + 4, :], pt[:],
                                 ACTF.Identity, bias=bias)
        else:
            pv = pt[:].rearrange("p (d i j) -> p d i j", d=4, i=NH)
            nc.scalar.activation(
                kvp[:, proj - 1, g, 4 * u:4 * u + 4, 1:1 + NH, 1:1 + NW], pv,
                ACTF.Identity, bias=bias)

    # ---------------- attention emitters ----------------
    def qk_pair(g, pair):
        """products (per-neighbor tiles) + PE d-reduction chain + exp"""
        tiles = []
        for n in pair:
            di, dj = n // 3, n % 3
            ph = php.tile([P, D, T], BF16, tag="php")
            eng = nc.gpsimd if n in POOL_QK else nc.vector
            eng.tensor_tensor(ph[:], qf[:, g],
                              kvp[:, 0, g, :, di:di + NH, dj:dj + NW],
                              op=ALU.mult)
            tiles.append(ph)
        lp = psL.tile([P, 512], F32, tag="psL")
        for w, ph in enumerate(tiles):
            for d in range(D):
                nc.tensor.matmul(lp[:, w * T:(w + 1) * T], ident,
                                 ph[:, d, :], start=(d == 0),
                                 stop=(d == D - 1))
        nc.scalar.activation(eslab[:, g, pair[0]:pair[0] + len(pair), :],
                             lp[:, :len(pair) * T], ACTF.Exp, scale=SCALE)

    def softmax_sums(g):
        """9-way sum on PE into PSUM, evacuated to SBUF by ACT"""
        sp = psL.tile([P, 512], F32, tag="psL")
        for n in range(9):
            nc.tensor.matmul(sp[:, :T], ident, eslab[:, g, n, :],
                             start=(n == 0), stop=(n == 8))
        nc.scalar.copy(ssb[:, g, :], sp[:, :T])

    def softmax_dve(g):
        """approx reciprocal, e <- e * (1/s)"""
        nc.vector.reciprocal_approx_fast(rrf[:, g, :], ssb[:, g, :])
        nc.scalar.copy(rr[:, g, :], rrf[:, g, :])
        ev = eslab[:, g]
        nc.vector.tensor_tensor(
            ev, ev, rr[:, g, :].unsqueeze(1).broadcast_to([P, 9, T]),
            op=ALU.mult)

    def av_products(g, hf):
        """p*V products for one d-half, per-neighbor tiles"""
        ds = slice(8 * hf, 8 * hf + 8)
        tiles = {}
        for n in list(range(9)):
            di, dj = n // 3, n % 3
            av = avp.tile([P, D // 2, T], BF16, tag="avs")
            pe = eslab[:, g, n, :].unsqueeze(1).broadcast_to([P, 8, T])
            eng = nc.gpsimd if n in POOL_AV[hf] else nc.vector
            eng.tensor_tensor(av[:], kvp[:, 1, g, ds, di:di + NH, dj:dj + NW],
                              pe, op=ALU.mult)
            tiles[n] = av
        return tiles

    def av_reduce(g, hf, tiles, enh):
        """accumulate over neighbors on PE in 4-d-plane quarters"""
        for q in range(2):
            acc = psA.tile([P, 1024], F32, tag="psA")
            for it, (n, av) in enumerate(sorted(tiles.items())):
                fl = av[:].rearrange("p d t -> p (d t)")
                for j in range(2):
                    cs = slice(q * 1024 + j * 512, q * 1024 + (j + 1) * 512)
                    nc.tensor.matmul(acc[:, j * 512:(j + 1) * 512], ident,
                                     fl[:, cs], start=(it == 0),
                                     stop=(it == 8))
            nc.scalar.activation(enh[:, 8 * hf + 4 * q:8 * hf + 4 * q + 4, :],
                                 acc[:], ACTF.Identity)

    def tail(g, enh, hf):
        cs = slice(hf * 2048, (hf + 1) * 2048)
        ot = outp.tile([P, PIX // 2], BF16, tag="outf")
        nc.vector.tensor_tensor(
            ot[:], enh[:].rearrange("p d t -> p (d t)")[:, cs],
            xb[:, g, cs], op=ALU.add)
        nc.sync.dma_start(out_d[:, g, cs], ot[:])

    # ---------------- schedule ----------------
    # group 0 Q and K projections first (QK products gate on them),
    # interleaved by chunk so both finish as early as the DMA allows
    for u in range(4):
        proj_unit(0, 0, u)
        proj_unit(1, 0, u)

    # remaining projections interleaved into the QK g0 PE stream:
    # Q1/K1 early (QK g1 products follow right after QK g0 on DVE), then V0, V1
    rest = ([(0, 1, u) for u in range(4)] + [(1, 1, u) for u in range(4)]
            + [(2, 0, u) for u in range(4)] + [(2, 1, u) for u in range(4)])

    splits = (0, 6, 12, 16, 16, 16)
    for i, pair in enumerate(PAIRS):
        qk_pair(0, pair)
        for pr, g, u in rest[splits[i]:splits[i + 1]]:
            proj_unit(pr, g, u)
    softmax_sums(0)
    for pair in PAIRS:
        qk_pair(1, pair)
    softmax_dve(0)
    softmax_sums(1)

    enh0 = enhp.tile([P, D, T], BF16, tag="enh")
    for hf in range(2):
        t0 = av_products(0, hf)
        av_reduce(0, hf, t0, enh0)
        tail(0, enh0, hf)
        if hf == 0:
            softmax_dve(1)
    enh1 = enhp.tile([P, D, T], BF16, tag="enh")
    for hf in range(2):
        t1 = av_products(1, hf)
        av_reduce(1, hf, t1, enh1)
        tail(1, enh1, hf)


def _build():
    global _BUILT
    if _BUILT is None:
        nc = bacc.Bacc(
            "TRN2", target_bir_lowering=False, debug=False, num_devices=N_CORES
        )
        with tile.TileContext(nc) as tc:
            with ExitStack() as ctx:
                _emit(ctx, tc)
        nc.compile()
        _BUILT = nc
    return _BUILT


def _tokenize(x: np.ndarray) -> np.ndarray:
    """[C,H,W] -> [C, D*T] d-major token order: index = (u,v,I,J)."""
    c = x.shape[0]
    return (
        x.reshape(c, NH, TS, NW, TS).transpose(0, 2, 4, 1, 3).reshape(c, PIX)
    )


def _untokenize(y: np.ndarray) -> np.ndarray:
    """[C, D*T] d-major token order -> [C, H, W]."""
    c = y.shape[0]
    return (
        y.reshape(c, TS, TS, NH, NW).transpose(0, 3, 1, 4, 2).reshape(c, H, W)
    )


def _part_fold(x: np.ndarray) -> np.ndarray:
    """[C, F] -> [P, C//P, F] partition-major fold."""
    return np.ascontiguousarray(
        x.reshape(C // P, P, -1).transpose(1, 0, 2)
    )


def _lhsT_blocks(wmat, dst, base, scale=1.0):
    """pack W[c_out, a_in] into per-(g,h) lhsT blocks: lhsT[p, c]=W[gP+c, hP+p]"""
    wm = np.asarray(wmat, np.float32) * scale
    for g in range(G):
        for h in range(2):
            blk = wm[g * P:(g + 1) * P, h * P:(h + 1) * P].T
            o = base + (g * 2 + h) * P
            dst[:, o:o + P] = blk


def _prep_maps(blue_feat, white_feat, Wq, bq, Wk, bk, Wv, bv):
    bf16 = ml_dtypes.bfloat16
    fp8 = mybir.dt.np(FP8)

    sm = np.zeros((P, SMW), np.float32)
    sm[:, SM_IDENT:SM_IDENT + P] = np.eye(P, dtype=np.float32)
    _lhsT_blocks(Wq, sm, SM_WQ)
    _lhsT_blocks(Wk, sm, SM_WK)
    _lhsT_blocks(Wv, sm, SM_WV)
    for g in range(G):
        sm[:, SM_BALL + g] = np.asarray(bq, np.float32)[g * P:(g + 1) * P]
        sm[:, SM_BALL + 2 + g] = np.asarray(bk, np.float32)[g * P:(g + 1) * P]
        sm[:, SM_BALL + 4 + g] = np.asarray(bv, np.float32)[g * P:(g + 1) * P]
    sm = sm.astype(bf16)

    maps = []
    for b in range(B):
        xbm = _part_fold(_tokenize(np.asarray(blue_feat[b], np.float32)))
        xwm = _part_fold(_tokenize(np.asarray(white_feat[b], np.float32)))
        maps.append({
            "xb": xbm.astype(bf16),
            "xw": xwm.astype(bf16),
            "sm": sm,
        })
    return maps


def _gather(results) -> np.ndarray:
    out = np.empty((B, C, H, W), np.float32)
    for b in range(B):
        y = results[b]["out"]  # [P, G, PIX] bf16
        y = np.asarray(y, np.float32).transpose(1, 0, 2).reshape(C, PIX)
        out[b] = _untokenize(y)
    return out


def _install_ntff_hook():
    """The agent image's antenv lacks axon_hooks; synthesize it so
    run_bass_kernel_spmd(trace=True) can drive NTFF profiling via the
    injected libaxon_pjrt.so C ABI (mirrors trn_agent_boot.trn_boot)."""
    import contextlib
    import ctypes
    import types

    if "antenv.axon_hooks" in sys.modules:
        return
    so_path = "/opt/axon/libaxon_pjrt.so"
    lib = ctypes.CDLL(so_path)
    if not hasattr(lib, "axon_start_nrt_profile"):
        return
    lib.axon_start_nrt_profile.argtypes = [
        ctypes.POINTER(ctypes.c_int64),
        ctypes.c_size_t,
    ]
    lib.axon_start_nrt_profile.restype = ctypes.c_int64
    lib.axon_stop_nrt_profile.argtypes = [ctypes.c_char_p]
    lib.axon_stop_nrt_profile.restype = ctypes.c_int64

    @contextlib.contextmanager
    def _hook(output_dir, device_ids):
        import jax

        jax.devices()
        if device_ids:
            ids = (ctypes.c_int64 * len(device_ids))(*device_ids)
            rc = lib.axon_start_nrt_profile(ids, len(device_ids))
        else:
            rc = lib.axon_start_nrt_profile(None, 0)
        if rc != 0:
            raise RuntimeError(f"axon_start_nrt_profile rc={rc}")
        try:
            yield
        finally:
            n = lib.axon_stop_nrt_profile(str(output_dir).encode())
            print(f"ntff profile: {n} file(s) written to {output_dir}")

    mod = types.ModuleType("antenv.axon_hooks")
    mod.get_axon_ntff_profile_hook = lambda: _hook  # type: ignore[attr-defined]
    mod.set_axon_ntff_profile_hook = lambda h: None  # type: ignore[attr-defined]
    sys.modules["antenv.axon_hooks"] = mod


def run(trace=False, **inputs):
    nc = _build()
    maps = _prep_maps(**inputs)
    if trace:
        _install_ntff_hook()
    res = run_bass_kernel_spmd(nc, maps, list(range(N_CORES)), trace=trace)
    return _gather(res.results), res


def kernel(**inputs) -> np.ndarray:
    out, _ = run(trace=False, **inputs)
    return out


# revision 26
# speedup vs baseline: 1.0975x; 1.0663x over previous
"""Trainium2 Bass kernel for nn_CrossModalAttention (sparse per-channel 3x3
token-window attention).

Contract: kernel(**inputs) takes the FULL fp32 inputs (B=8,C=256,H=W=64) and
returns the FULL fp32 output.  Internally: data-parallel over batch across the
8 NeuronCores (1 batch element per core), params replicated.

v3 design (calibrated against the fake-NRT cost model):
  - K projection runs as fp8e4 DoubleRow matmuls: one 256-deep contraction
    pass per 512-pixel chunk (half the PE cycles of bf16).  Weights are
    pre-scaled by 16 on the host to dodge fp8 subnormals; the PSUM evacuation
    un-scales (ACT activation scale=1/16) and fuses the bias + bf16 cast.
    Q and V stay bf16 (V errors hit the output linearly; Q feeds the
    residual-critical logits).
  - QK/AV elementwise products are bf16 TensorTensor ops (the only DVE op
    with a 2x mode here), split between DVE and GpSimd(Pool): Pool takes the
    late-consumed neighbors so it never sits on the softmax critical path.
  - PE reduces products with identity-matmul accumulation chains; the 9-way
    softmax denominator is also a PE identity chain into PSUM.
  - reciprocal_approx_fast on DVE; one fat 2x e*(1/s) multiply per group.
  - Residual add + store in bf16 (host casts back to fp32).

Layout: activations are d-major [c, d, t]; d = pixel-within-token (16),
t = (I,J) token index (256).  K/V live in zero-padded 18x18 token grids so
all 9 neighbor views are plain strided APs (this simulator's DVE 2x mode has
no offset-parity constraint).
"""

import os
import sys
from contextlib import ExitStack

import numpy as np

for _p in ("/opt/trn_rl_repo",):
    if _p not in sys.path and os.path.isdir(_p):
        sys.path.insert(0, _p)

import ml_dtypes  # noqa: E402

import concourse.bacc as bacc  # noqa: E402
import concourse.bass as bass  # noqa: E402
import concourse.tile as tile  # noqa: E402
from concourse import mybir  # noqa: E402
from concourse.bass_utils import run_bass_kernel_spmd  # noqa: E402

BF16 = mybir.dt.bfloat16
F32 = mybir.dt.float32
FP8 = mybir.dt.float8e4
ALU = mybir.AluOpType
ACTF = mybir.ActivationFunctionType
DR = mybir.MatmulPerfMode.DoubleRow

B, C, H, W = 8, 256, 64, 64
TS = 4                      # token size
NH = H // TS                # 16 token rows
NW = W // TS                # 16 token cols
T = NH * NW                 # 256 tokens
D = TS * TS                 # 16 pixels per token
G = 2                       # channel groups of 128
P = 128
PIX = H * W                 # 4096
GRID = NH + 2               # 18 (zero-padded token grid)
SCALE = float(D) ** -0.5    # 0.25
N_CORES = 8
WSCALE = 16.0               # fp8 weight pre-scale (undone at evacuation)

# smalls layout (bf16): ident | Wq lhsT | Wk lhsT | Wv lhsT | biases
SM_IDENT = 0
SM_WQ = P                       # (g,h) blocks of 128 -> 512
SM_WK = SM_WQ + 4 * P
SM_WV = SM_WK + 4 * P
SM_BALL = SM_WV + 4 * P         # 6 cols (proj*2+g, proj order Q,K,V)
SMW = SM_BALL + 6

PAIRS = ((8,), (0, 1), (2, 3), (4, 5), (6, 7))
POOL_QK = {8}         # QK neighbor products emitted on Pool (both groups)
POOL_AV = {0: {7, 8}, 1: {8}}   # per-half AV pool assignment

_BUILT = None


def _emit(ctx: ExitStack, tc: "tile.TileContext"):
    nc = tc.nc

    xb_d = nc.dram_tensor("xb", [P, G, PIX], BF16, kind="ExternalInput").ap()
    xw_d = nc.dram_tensor("xw", [P, G, PIX], BF16, kind="ExternalInput").ap()
    sm_d = nc.dram_tensor("sm", [P, SMW], BF16, kind="ExternalInput").ap()
    out_d = nc.dram_tensor("out", [P, G, PIX], BF16, kind="ExternalOutput").ap()

    consts = ctx.enter_context(tc.tile_pool(name="consts", bufs=1))
    php = ctx.enter_context(tc.tile_pool(name="php", bufs=4))
    avp = ctx.enter_context(tc.tile_pool(name="avp", bufs=11))
    enhp = ctx.enter_context(tc.tile_pool(name="enhp", bufs=2))
    outp = ctx.enter_context(tc.tile_pool(name="outp", bufs=2))
    psP = ctx.enter_context(tc.tile_pool(name="psP", bufs=2, space="PSUM"))
    psL = ctx.enter_context(tc.tile_pool(name="psL", bufs=2, space="PSUM"))
    psA = ctx.enter_context(tc.tile_pool(name="psA", bufs=2, space="PSUM"))

    xb = consts.tile([P, G, PIX], BF16)
    xw = consts.tile([P, G, PIX], BF16)
    sm = consts.tile([P, SMW], BF16)
    qf = consts.tile([P, G, D, T], BF16)
    kvp = consts.tile([P, 2, G, D, GRID, GRID], BF16)  # [kv, g, d, I, J]
    eslab = consts.tile([P, G, 9, T], BF16)
    rr = consts.tile([P, G, T], BF16)
    rrf = consts.tile([P, G, T], F32)
    ssb = consts.tile([P, G, T], F32)

    nc.sync.dma_start(sm[:], sm_d[:])
    nc.sync.dma_start(xb[:], xb_d[:])
    nc.scalar.dma_start(xw[:], xw_d[:])

    ident = sm[:, SM_IDENT:SM_IDENT + P]
    wq = sm[:, SM_WQ:SM_WQ + 4 * P].rearrange("p (g h c) -> p g h c", g=G, h=2)
    wk = sm[:, SM_WK:SM_WK + 4 * P].rearrange("p (g h c) -> p g h c", g=G, h=2)
    wv = sm[:, SM_WV:SM_WV + 4 * P].rearrange("p (g h c) -> p g h c", g=G, h=2)
    ball = sm[:, SM_BALL:SM_BALL + 6]

    # zero the padding ring of the K/V token grids (rows and cols 0,17).
    for kv in range(2):
        for g in range(G):
            nc.vector.memset(kvp[:, kv, g, :, 0:GRID:GRID - 1, :], 0.0)
            nc.vector.memset(kvp[:, kv, g, :, :, 0:GRID:GRID - 1], 0.0)

    # ---------------- projection emitters ----------------
    def proj_unit(proj, g, u):
        """one 1024-pixel chunk of a bf16 projection + evacuation.
        proj: 0=Q (from xb -> qf), 1=K (xw -> grid 0), 2=V (xw -> grid 1)."""
        w = (wq, wk, wv)[proj]
        src = xb if proj == 0 else xw
        bias = ball[:, 2 * proj + g:2 * proj + g + 1]
        for j in range(2):
            pt = psP.tile([P, 512], F32, tag="psP")
            cols = slice(u * 1024 + j * 512, u * 1024 + (j + 1) * 512)
            nc.tensor.matmul(pt[:], w[:, g, 0], src[:, 0, cols],
                             start=True, stop=False)
            nc.tensor.matmul(pt[:], w[:, g, 1], src[:, 1, cols],
                             start=False, stop=True)
            dlo = 4 * u + 2 * j
            if proj == 0:
                nc.scalar.activation(qf[:, g, dlo:dlo + 2, :], pt[:],
                                     ACTF.Identity, bias=bias)
            else:
                pv = pt[:].rearrange("p (d i j) -> p d i j", d=2, i=NH)
                nc.scalar.activation(
                    kvp[:, proj - 1, g, dlo:dlo + 2, 1:1 + NH, 1:1 + NW], pv,
                    ACTF.Identity, bias=bias)

    # ---------------- attention emitters ----------------
    def qk_pair(g, pair):
        """products (per-neighbor tiles) + PE d-reduction chain + exp"""
        tiles = []
        for n in pair:
            di, dj = n // 3, n % 3
            ph = php.tile([P, D, T], BF16, tag="php")
            eng = nc.gpsimd if n in POOL_QK else nc.vector
            eng.tensor_tensor(ph[:], qf[:, g],
                              kvp[:, 0, g, :, di:di + NH, dj:dj + NW],
                              op=ALU.mult)
            tiles.append(ph)
        lp = psL.tile([P, 512], F32, tag="psL")
        for w, ph in enumerate(tiles):
            for d in range(D):
                nc.tensor.matmul(lp[:, w * T:(w + 1) * T], ident,
                                 ph[:, d, :], start=(d == 0),
                                 stop=(d == D - 1))
        nc.scalar.activation(eslab[:, g, pair[0]:pair[0] + len(pair), :],
                             lp[:, :len(pair) * T], ACTF.Exp, scale=SCALE)

    def softmax_sums(g):
        """9-way sum on PE into PSUM, evacuated to SBUF by ACT"""
        sp = psL.tile([P, 512], F32, tag="psL")
        for n in range(9):
            nc.tensor.matmul(sp[:, :T], ident, eslab[:, g, n, :],
                             start=(n == 0), stop=(n == 8))
        nc.scalar.copy(ssb[:, g, :], sp[:, :T])

    def softmax_dve(g):
        """approx reciprocal, e <- e * (1/s)"""
        nc.vector.reciprocal_approx_fast(rrf[:, g, :], ssb[:, g, :])
        nc.scalar.copy(rr[:, g, :], rrf[:, g, :])
        ev = eslab[:, g]
        nc.vector.tensor_tensor(
            ev, ev, rr[:, g, :].unsqueeze(1).broadcast_to([P, 9, T]),
            op=ALU.mult)

    def av_products(g, hf):
        """p*V products for one d-half, per-neighbor tiles"""
        ds = slice(8 * hf, 8 * hf + 8)
        tiles = {}
        for n in list(range(9)):
            di, dj = n // 3, n % 3
            av = avp.tile([P, D // 2, T], BF16, tag="avs")
            pe = eslab[:, g, n, :].unsqueeze(1).broadcast_to([P, 8, T])
            eng = nc.gpsimd if n in POOL_AV[hf] else nc.vector
            eng.tensor_tensor(av[:], kvp[:, 1, g, ds, di:di + NH, dj:dj + NW],
                              pe, op=ALU.mult)
            tiles[n] = av
        return tiles

    def av_reduce(g, hf, tiles, enh):
        """accumulate over neighbors on PE in 4-d-plane quarters"""
        for q in range(2):
            acc = psA.tile([P, 1024], F32, tag="psA")
            for it, (n, av) in enumerate(sorted(tiles.items())):
                fl = av[:].rearrange("p d t -> p (d t)")
                for j in range(2):
                    cs = slice(q * 1024 + j * 512, q * 1024 + (j + 1) * 512)
                    nc.tensor.matmul(acc[:, j * 512:(j + 1) * 512], ident,
                                     fl[:, cs], start=(it == 0),
                                     stop=(it == 8))
            nc.scalar.activation(enh[:, 8 * hf + 4 * q:8 * hf + 4 * q + 4, :],
                                 acc[:], ACTF.Identity)

    def tail(g, enh, hf):
        cs = slice(hf * 2048, (hf + 1) * 2048)
        ot = outp.tile([P, PIX // 2], BF16, tag="outf")
        nc.vector.tensor_tensor(
            ot[:], enh[:].rearrange("p d t -> p (d t)")[:, cs],
            xb[:, g, cs], op=ALU.add)
        nc.sync.dma_start(out_d[:, g, cs], ot[:])

    # ---------------- schedule ----------------
    # group 0 Q and K projections first (QK products gate on them),
    # interleaved by chunk so both finish as early as the DMA allows
    for u in range(4):
        proj_unit(0, 0, u)
        proj_unit(1, 0, u)

    # remaining projections interleaved into the QK g0 PE stream:
    # Q1/K1 early (QK g1 products follow right after QK g0 on DVE), then V0, V1
    rest = ([(0, 1, u) for u in range(4)] + [(1, 1, u) for u in range(4)]
            + [(2, 0, u) for u in range(4)] + [(2, 1, u) for u in range(4)])

    for i, pair in enumerate(PAIRS):
        qk_pair(0, pair)
        for pr, g, u in rest[i * 3:i * 3 + 3]:
            proj_unit(pr, g, u)
    softmax_sums(0)
    softmax_dve(0)
    for i, pair in enumerate(PAIRS):
        qk_pair(1, pair)
        for pr, g, u in rest[15 + i:16 + i]:
            proj_unit(pr, g, u)
    softmax_sums(1)
    softmax_dve(1)

    enh0 = enhp.tile([P, D, T], BF16, tag="enh")
    for hf in range(2):
        t0 = av_products(0, hf)
        av_reduce(0, hf, t0, enh0)
        tail(0, enh0, hf)
    enh1 = enhp.tile([P, D, T], BF16, tag="enh")
    for hf in range(2):
        t1 = av_products(1, hf)
        av_reduce(1, hf, t1, enh1)
        tail(1, enh1, hf)


def _build():
    global _BUILT
    if _BUILT is None:
        nc = bacc.Bacc(
            "TRN2", target_bir_lowering=False, debug=False, num_devices=N_CORES
        )
        with tile.TileContext(nc) as tc:
            with ExitStack() as ctx:
                _emit(ctx, tc)
        nc.compile()
        _BUILT = nc
    return _BUILT


def _tokenize(x: np.ndarray) -> np.ndarray:
    """[C,H,W] -> [C, D*T] d-major token order: index = (u,v,I,J)."""
    c = x.shape[0]
    return (
        x.reshape(c, NH, TS, NW, TS).transpose(0, 2, 4, 1, 3).reshape(c, PIX)
    )


def _untokenize(y: np.ndarray) -> np.ndarray:
    """[C, D*T] d-major token order -> [C, H, W]."""
    c = y.shape[0]
    return (
        y.reshape(c, TS, TS, NH, NW).transpose(0, 3, 1, 4, 2).reshape(c, H, W)
    )


def _part_fold(x: np.ndarray) -> np.ndarray:
    """[C, F] -> [P, C//P, F] partition-major fold."""
    return np.ascontiguousarray(
        x.reshape(C // P, P, -1).transpose(1, 0, 2)
    )


def _lhsT_blocks(wmat, dst, base, scale=1.0):
    """pack W[c_out, a_in] into per-(g,h) lhsT blocks: lhsT[p, c]=W[gP+c, hP+p]"""
    wm = np.asarray(wmat, np.float32) * scale
    for g in range(G):
        for h in range(2):
            blk = wm[g * P:(g + 1) * P, h * P:(h + 1) * P].T
            o = base + (g * 2 + h) * P
            dst[:, o:o + P] = blk


def _prep_maps(blue_feat, white_feat, Wq, bq, Wk, bk, Wv, bv):
    bf16 = ml_dtypes.bfloat16
    fp8 = mybir.dt.np(FP8)

    sm = np.zeros((P, SMW), np.float32)
    sm[:, SM_IDENT:SM_IDENT + P] = np.eye(P, dtype=np.float32)
    _lhsT_blocks(Wq, sm, SM_WQ)
    _lhsT_blocks(Wk, sm, SM_WK)
    _lhsT_blocks(Wv, sm, SM_WV)
    for g in range(G):
        sm[:, SM_BALL + g] = np.asarray(bq, np.float32)[g * P:(g + 1) * P]
        sm[:, SM_BALL + 2 + g] = np.asarray(bk, np.float32)[g * P:(g + 1) * P]
        sm[:, SM_BALL + 4 + g] = np.asarray(bv, np.float32)[g * P:(g + 1) * P]
    sm = sm.astype(bf16)

    maps = []
    for b in range(B):
        xbm = _part_fold(_tokenize(np.asarray(blue_feat[b], np.float32)))
        xwm = _part_fold(_tokenize(np.asarray(white_feat[b], np.float32)))
        maps.append({
            "xb": xbm.astype(bf16),
            "xw": xwm.astype(bf16),
            "sm": sm,
        })
    return maps


def _gather(results) -> np.ndarray:
    out = np.empty((B, C, H, W), np.float32)
    for b in range(B):
        y = results[b]["out"]  # [P, G, PIX] bf16
        y = np.asarray(y, np.float32).transpose(1, 0, 2).reshape(C, PIX)
        out[b] = _untokenize(y)
    return out


def _install_ntff_hook():
    """The agent image's antenv lacks axon_hooks; synthesize it so
    run_bass_kernel_spmd(trace=True) can drive NTFF profiling via the
    injected libaxon_pjrt.so C ABI (mirrors trn_agent_boot.trn_boot)."""
    import contextlib
    import ctypes
    import types

    if "antenv.axon_hooks" in sys.modules:
        return
    so_path = "/opt/axon/libaxon_pjrt.so"
    lib = ctypes.CDLL(so_path)
    if not hasattr(lib, "axon_start_nrt_profile"):
        return
    lib.axon_start_nrt_profile.argtypes = [
        ctypes.POINTER(ctypes.c_int64),
        ctypes.c_size_t,
    ]
    lib.axon_start_nrt_profile.restype = ctypes.c_int64
    lib.axon_stop_nrt_profile.argtypes = [ctypes.c_char_p]
    lib.axon_stop_nrt_profile.restype = ctypes.c_int64

    @contextlib.contextmanager
    def _hook(output_dir, device_ids):
        import jax

        jax.devices()
        if device_ids:
            ids = (ctypes.c_int64 * len(device_ids))(*device_ids)
            rc = lib.axon_start_nrt_profile(ids, len(device_ids))
        else:
            rc = lib.axon_start_nrt_profile(None, 0)
        if rc != 0:
            raise RuntimeError(f"axon_start_nrt_profile rc={rc}")
        try:
            yield
        finally:
            n = lib.axon_stop_nrt_profile(str(output_dir).encode())
            print(f"ntff profile: {n} file(s) written to {output_dir}")

    mod = types.ModuleType("antenv.axon_hooks")
    mod.get_axon_ntff_profile_hook = lambda: _hook  # type: ignore[attr-defined]
    mod.set_axon_ntff_profile_hook = lambda h: None  # type: ignore[attr-defined]
    sys.modules["antenv.axon_hooks"] = mod


def run(trace=False, **inputs):
    nc = _build()
    maps = _prep_maps(**inputs)
    if trace:
        _install_ntff_hook()
    res = run_bass_kernel_spmd(nc, maps, list(range(N_CORES)), trace=trace)
    return _gather(res.results), res


def kernel(**inputs) -> np.ndarray:
    out, _ = run(trace=False, **inputs)
    return out


# revision 27
# speedup vs baseline: 1.1088x; 1.0103x over previous
"""Trainium2 Bass kernel for nn_CrossModalAttention (sparse per-channel 3x3
token-window attention).

Contract: kernel(**inputs) takes the FULL fp32 inputs (B=8,C=256,H=W=64) and
returns the FULL fp32 output.  Internally: data-parallel over batch across the
8 NeuronCores (1 batch element per core), params replicated.

v3 design (calibrated against the fake-NRT cost model):
  - K projection runs as fp8e4 DoubleRow matmuls: one 256-deep contraction
    pass per 512-pixel chunk (half the PE cycles of bf16).  Weights are
    pre-scaled by 16 on the host to dodge fp8 subnormals; the PSUM evacuation
    un-scales (ACT activation scale=1/16) and fuses the bias + bf16 cast.
    Q and V stay bf16 (V errors hit the output linearly; Q feeds the
    residual-critical logits).
  - QK/AV elementwise products are bf16 TensorTensor ops (the only DVE op
    with a 2x mode here), split between DVE and GpSimd(Pool): Pool takes the
    late-consumed neighbors so it never sits on the softmax critical path.
  - PE reduces products with identity-matmul accumulation chains; the 9-way
    softmax denominator is also a PE identity chain into PSUM.
  - reciprocal_approx_fast on DVE; one fat 2x e*(1/s) multiply per group.
  - Residual add + store in bf16 (host casts back to fp32).

Layout: activations are d-major [c, d, t]; d = pixel-within-token (16),
t = (I,J) token index (256).  K/V live in zero-padded 18x18 token grids so
all 9 neighbor views are plain strided APs (this simulator's DVE 2x mode has
no offset-parity constraint).
"""

import os
import sys
from contextlib import ExitStack

import numpy as np

for _p in ("/opt/trn_rl_repo",):
    if _p not in sys.path and os.path.isdir(_p):
        sys.path.insert(0, _p)

import ml_dtypes  # noqa: E402

import concourse.bacc as bacc  # noqa: E402
import concourse.bass as bass  # noqa: E402
import concourse.tile as tile  # noqa: E402
from concourse import mybir  # noqa: E402
from concourse.bass_utils import run_bass_kernel_spmd  # noqa: E402

BF16 = mybir.dt.bfloat16
F32 = mybir.dt.float32
FP8 = mybir.dt.float8e4
ALU = mybir.AluOpType
ACTF = mybir.ActivationFunctionType
DR = mybir.MatmulPerfMode.DoubleRow

B, C, H, W = 8, 256, 64, 64
TS = 4                      # token size
NH = H // TS                # 16 token rows
NW = W // TS                # 16 token cols
T = NH * NW                 # 256 tokens
D = TS * TS                 # 16 pixels per token
G = 2                       # channel groups of 128
P = 128
PIX = H * W                 # 4096
GRID = NH + 2               # 18 (zero-padded token grid)
SCALE = float(D) ** -0.5    # 0.25
N_CORES = 8
WSCALE = 16.0               # fp8 weight pre-scale (undone at evacuation)

# smalls layout (bf16): ident | Wq lhsT | Wk lhsT | Wv lhsT | biases
SM_IDENT = 0
SM_WQ = P                       # (g,h) blocks of 128 -> 512
SM_WK = SM_WQ + 4 * P
SM_WV = SM_WK + 4 * P
SM_BALL = SM_WV + 4 * P         # 6 cols (proj*2+g, proj order Q,K,V)
SMW = SM_BALL + 6

PAIRS = ((0, 1), (2, 3), (4, 5), (6, 7), (8,))
POOL_QK = {8}         # QK neighbor products emitted on Pool (both groups)
POOL_AV = {0: {7, 8}, 1: {8}}   # per-half AV pool assignment

_BUILT = None


def _emit(ctx: ExitStack, tc: "tile.TileContext"):
    nc = tc.nc

    xb_d = nc.dram_tensor("xb", [P, G, PIX], BF16, kind="ExternalInput").ap()
    xw_d = nc.dram_tensor("xw", [P, G, PIX], BF16, kind="ExternalInput").ap()
    sm_d = nc.dram_tensor("sm", [P, SMW], BF16, kind="ExternalInput").ap()
    out_d = nc.dram_tensor("out", [P, G, PIX], BF16, kind="ExternalOutput").ap()

    consts = ctx.enter_context(tc.tile_pool(name="consts", bufs=1))
    php = ctx.enter_context(tc.tile_pool(name="php", bufs=4))
    avp = ctx.enter_context(tc.tile_pool(name="avp", bufs=11))
    enhp = ctx.enter_context(tc.tile_pool(name="enhp", bufs=2))
    outp = ctx.enter_context(tc.tile_pool(name="outp", bufs=2))
    psP = ctx.enter_context(tc.tile_pool(name="psP", bufs=2, space="PSUM"))
    psL = ctx.enter_context(tc.tile_pool(name="psL", bufs=2, space="PSUM"))
    psA = ctx.enter_context(tc.tile_pool(name="psA", bufs=2, space="PSUM"))

    xb = consts.tile([P, G, PIX], BF16)
    xw = consts.tile([P, G, PIX], BF16)
    sm = consts.tile([P, SMW], BF16)
    qf = consts.tile([P, G, D, T], BF16)
    kvp = consts.tile([P, 2, G, D, GRID, GRID], BF16)  # [kv, g, d, I, J]
    eslab = consts.tile([P, G, 9, T], BF16)
    rr = consts.tile([P, G, T], BF16)
    rrf = consts.tile([P, G, T], F32)
    ssb = consts.tile([P, G, T], F32)

    nc.sync.dma_start(sm[:], sm_d[:])
    nc.sync.dma_start(xb[:], xb_d[:])
    nc.scalar.dma_start(xw[:], xw_d[:])

    ident = sm[:, SM_IDENT:SM_IDENT + P]
    wq = sm[:, SM_WQ:SM_WQ + 4 * P].rearrange("p (g h c) -> p g h c", g=G, h=2)
    wk = sm[:, SM_WK:SM_WK + 4 * P].rearrange("p (g h c) -> p g h c", g=G, h=2)
    wv = sm[:, SM_WV:SM_WV + 4 * P].rearrange("p (g h c) -> p g h c", g=G, h=2)
    ball = sm[:, SM_BALL:SM_BALL + 6]

    # zero the padding ring of the K/V token grids (rows and cols 0,17).
    for kv in range(2):
        for g in range(G):
            nc.vector.memset(kvp[:, kv, g, :, 0:GRID:GRID - 1, :], 0.0)
            nc.vector.memset(kvp[:, kv, g, :, :, 0:GRID:GRID - 1], 0.0)

    # ---------------- projection emitters ----------------
    def proj_unit(proj, g, u):
        """one 1024-pixel chunk of a bf16 projection + evacuation.
        proj: 0=Q (from xb -> qf), 1=K (xw -> grid 0), 2=V (xw -> grid 1)."""
        w = (wq, wk, wv)[proj]
        src = xb if proj == 0 else xw
        bias = ball[:, 2 * proj + g:2 * proj + g + 1]
        for j in range(2):
            pt = psP.tile([P, 512], F32, tag="psP")
            cols = slice(u * 1024 + j * 512, u * 1024 + (j + 1) * 512)
            nc.tensor.matmul(pt[:], w[:, g, 0], src[:, 0, cols],
                             start=True, stop=False)
            nc.tensor.matmul(pt[:], w[:, g, 1], src[:, 1, cols],
                             start=False, stop=True)
            dlo = 4 * u + 2 * j
            if proj == 0:
                nc.scalar.activation(qf[:, g, dlo:dlo + 2, :], pt[:],
                                     ACTF.Identity, bias=bias)
            else:
                pv = pt[:].rearrange("p (d i j) -> p d i j", d=2, i=NH)
                nc.scalar.activation(
                    kvp[:, proj - 1, g, dlo:dlo + 2, 1:1 + NH, 1:1 + NW], pv,
                    ACTF.Identity, bias=bias)

    # ---------------- attention emitters ----------------
    def qk_pair(g, pair):
        """products (per-neighbor tiles) + PE d-reduction chain + exp"""
        tiles = []
        for n in pair:
            di, dj = n // 3, n % 3
            ph = php.tile([P, D, T], BF16, tag="php")
            eng = nc.gpsimd if n in POOL_QK else nc.vector
            eng.tensor_tensor(ph[:], qf[:, g],
                              kvp[:, 0, g, :, di:di + NH, dj:dj + NW],
                              op=ALU.mult)
            tiles.append(ph)
        lp = psL.tile([P, 512], F32, tag="psL")
        for w, ph in enumerate(tiles):
            for d in range(D):
                nc.tensor.matmul(lp[:, w * T:(w + 1) * T], ident,
                                 ph[:, d, :], start=(d == 0),
                                 stop=(d == D - 1))
        nc.scalar.activation(eslab[:, g, pair[0]:pair[0] + len(pair), :],
                             lp[:, :len(pair) * T], ACTF.Exp, scale=SCALE)

    def softmax_sums(g):
        """9-way sum on PE into PSUM, evacuated to SBUF by ACT"""
        sp = psL.tile([P, 512], F32, tag="psL")
        for n in range(9):
            nc.tensor.matmul(sp[:, :T], ident, eslab[:, g, n, :],
                             start=(n == 0), stop=(n == 8))
        nc.scalar.copy(ssb[:, g, :], sp[:, :T])

    def softmax_dve(g):
        """approx reciprocal, e <- e * (1/s)"""
        nc.vector.reciprocal_approx_fast(rrf[:, g, :], ssb[:, g, :])
        nc.scalar.copy(rr[:, g, :], rrf[:, g, :])
        ev = eslab[:, g]
        nc.vector.tensor_tensor(
            ev, ev, rr[:, g, :].unsqueeze(1).broadcast_to([P, 9, T]),
            op=ALU.mult)

    def av_products(g, hf):
        """p*V products for one d-half, per-neighbor tiles"""
        ds = slice(8 * hf, 8 * hf + 8)
        tiles = {}
        for n in list(range(9)):
            di, dj = n // 3, n % 3
            av = avp.tile([P, D // 2, T], BF16, tag="avs")
            pe = eslab[:, g, n, :].unsqueeze(1).broadcast_to([P, 8, T])
            eng = nc.gpsimd if n in POOL_AV[hf] else nc.vector
            eng.tensor_tensor(av[:], kvp[:, 1, g, ds, di:di + NH, dj:dj + NW],
                              pe, op=ALU.mult)
            tiles[n] = av
        return tiles

    def av_reduce(g, hf, tiles, enh):
        """accumulate over neighbors on PE in 4-d-plane quarters"""
        for q in range(2):
            acc = psA.tile([P, 1024], F32, tag="psA")
            for it, (n, av) in enumerate(sorted(tiles.items())):
                fl = av[:].rearrange("p d t -> p (d t)")
                for j in range(2):
                    cs = slice(q * 1024 + j * 512, q * 1024 + (j + 1) * 512)
                    nc.tensor.matmul(acc[:, j * 512:(j + 1) * 512], ident,
                                     fl[:, cs], start=(it == 0),
                                     stop=(it == 8))
            nc.scalar.activation(enh[:, 8 * hf + 4 * q:8 * hf + 4 * q + 4, :],
                                 acc[:], ACTF.Identity)

    def tail(g, enh, hf):
        cs = slice(hf * 2048, (hf + 1) * 2048)
        ot = outp.tile([P, PIX // 2], BF16, tag="outf")
        nc.vector.tensor_tensor(
            ot[:], enh[:].rearrange("p d t -> p (d t)")[:, cs],
            xb[:, g, cs], op=ALU.add)
        nc.sync.dma_start(out_d[:, g, cs], ot[:])

    # ---------------- schedule ----------------
    # group 0 Q and K projections first (QK products gate on them),
    # interleaved by chunk so both finish as early as the DMA allows
    for u in range(4):
        proj_unit(0, 0, u)
        proj_unit(1, 0, u)

    # remaining projections interleaved into the QK g0 PE stream:
    # Q1/K1 early (QK g1 products follow right after QK g0 on DVE), then V0, V1
    rest = ([(0, 1, u) for u in range(4)] + [(1, 1, u) for u in range(4)]
            + [(2, 0, u) for u in range(4)] + [(2, 1, u) for u in range(4)])

    for i, pair in enumerate(PAIRS):
        qk_pair(0, pair)
        for pr, g, u in rest[i * 3:i * 3 + 3]:
            proj_unit(pr, g, u)
    softmax_sums(0)
    for i, pair in enumerate(PAIRS):
        qk_pair(1, pair)
        for pr, g, u in rest[15 + i:16 + i]:
            proj_unit(pr, g, u)
    softmax_dve(0)
    softmax_sums(1)

    enh0 = enhp.tile([P, D, T], BF16, tag="enh")
    for hf in range(2):
        t0 = av_products(0, hf)
        if hf == 0:
            softmax_dve(1)
        av_reduce(0, hf, t0, enh0)
        tail(0, enh0, hf)
    enh1 = enhp.tile([P, D, T], BF16, tag="enh")
    for hf in range(2):
        t1 = av_products(1, hf)
        av_reduce(1, hf, t1, enh1)
        tail(1, enh1, hf)


def _build():
    global _BUILT
    if _BUILT is None:
        nc = bacc.Bacc(
            "TRN2", target_bir_lowering=False, debug=False, num_devices=N_CORES
        )
        with tile.TileContext(nc) as tc:
            with ExitStack() as ctx:
                _emit(ctx, tc)
        nc.compile()
        _BUILT = nc
    return _BUILT


def _tokenize(x: np.ndarray) -> np.ndarray:
    """[C,H,W] -> [C, D*T] d-major token order: index = (u,v,I,J)."""
    c = x.shape[0]
    return (
        x.reshape(c, NH, TS, NW, TS).transpose(0, 2, 4, 1, 3).reshape(c, PIX)
    )


def _untokenize(y: np.ndarray) -> np.ndarray:
    """[C, D*T] d-major token order -> [C, H, W]."""
    c = y.shape[0]
    return (
        y.reshape(c, TS, TS, NH, NW).transpose(0, 3, 1, 4, 2).reshape(c, H, W)
    )


def _part_fold(x: np.ndarray) -> np.ndarray:
    """[C, F] -> [P, C//P, F] partition-major fold."""
    return np.ascontiguousarray(
        x.reshape(C // P, P, -1).transpose(1, 0, 2)
    )


def _lhsT_blocks(wmat, dst, base, scale=1.0):
    """pack W[c_out, a_in] into per-(g,h) lhsT blocks: lhsT[p, c]=W[gP+c, hP+p]"""
    wm = np.asarray(wmat, np.float32) * scale
    for g in range(G):
        for h in range(2):
            blk = wm[g * P:(g + 1) * P, h * P:(h + 1) * P].T
            o = base + (g * 2 + h) * P
            dst[:, o:o + P] = blk


def _prep_maps(blue_feat, white_feat, Wq, bq, Wk, bk, Wv, bv):
    bf16 = ml_dtypes.bfloat16
    fp8 = mybir.dt.np(FP8)

    sm = np.zeros((P, SMW), np.float32)
    sm[:, SM_IDENT:SM_IDENT + P] = np.eye(P, dtype=np.float32)
    _lhsT_blocks(Wq, sm, SM_WQ)
    _lhsT_blocks(Wk, sm, SM_WK)
    _lhsT_blocks(Wv, sm, SM_WV)
    for g in range(G):
        sm[:, SM_BALL + g] = np.asarray(bq, np.float32)[g * P:(g + 1) * P]
        sm[:, SM_BALL + 2 + g] = np.asarray(bk, np.float32)[g * P:(g + 1) * P]
        sm[:, SM_BALL + 4 + g] = np.asarray(bv, np.float32)[g * P:(g + 1) * P]
    sm = sm.astype(bf16)

    maps = []
    for b in range(B):
        xbm = _part_fold(_tokenize(np.asarray(blue_feat[b], np.float32)))
        xwm = _part_fold(_tokenize(np.asarray(white_feat[b], np.float32)))
        maps.append({
            "xb": xbm.astype(bf16),
            "xw": xwm.astype(bf16),
            "sm": sm,
        })
    return maps


def _gather(results) -> np.ndarray:
    out = np.empty((B, C, H, W), np.float32)
    for b in range(B):
        y = results[b]["out"]  # [P, G, PIX] bf16
        y = np.asarray(y, np.float32).transpose(1, 0, 2).reshape(C, PIX)
        out[b] = _untokenize(y)
    return out


def _install_ntff_hook():
    """The agent image's antenv lacks axon_hooks; synthesize it so
    run_bass_kernel_spmd(trace=True) can drive NTFF profiling via the
    injected libaxon_pjrt.so C ABI (mirrors trn_agent_boot.trn_boot)."""
    import contextlib
    import ctypes
    import types

    if "antenv.axon_hooks" in sys.modules:
        return
    so_path = "/opt/axon/libaxon_pjrt.so"
    lib = ctypes.CDLL(so_path)
    if not hasattr(lib, "axon_start_nrt_profile"):
        return
    lib.axon_start_nrt_profile.argtypes = [
        ctypes.POINTER(ctypes.c_int64),
        ctypes.c_size_t,
    ]
    lib.axon_start_nrt_profile.restype = ctypes.c_int64
    lib.axon_stop_nrt_profile.argtypes = [ctypes.c_char_p]
    lib.axon_stop_nrt_profile.restype = ctypes.c_int64

    @contextlib.contextmanager
    def _hook(output_dir, device_ids):
        import jax

        jax.devices()
        if device_ids:
            ids = (ctypes.c_int64 * len(device_ids))(*device_ids)
            rc = lib.axon_start_nrt_profile(ids, len(device_ids))
        else:
            rc = lib.axon_start_nrt_profile(None, 0)
        if rc != 0:
            raise RuntimeError(f"axon_start_nrt_profile rc={rc}")
        try:
            yield
        finally:
            n = lib.axon_stop_nrt_profile(str(output_dir).encode())
            print(f"ntff profile: {n} file(s) written to {output_dir}")

    mod = types.ModuleType("antenv.axon_hooks")
    mod.get_axon_ntff_profile_hook = lambda: _hook  # type: ignore[attr-defined]
    mod.set_axon_ntff_profile_hook = lambda h: None  # type: ignore[attr-defined]
    sys.modules["antenv.axon_hooks"] = mod


def run(trace=False, **inputs):
    nc = _build()
    maps = _prep_maps(**inputs)
    if trace:
        _install_ntff_hook()
    res = run_bass_kernel_spmd(nc, maps, list(range(N_CORES)), trace=trace)
    return _gather(res.results), res


def kernel(**inputs) -> np.ndarray:
    out, _ = run(trace=False, **inputs)
    return out
